# revision 20
# baseline (speedup 1.0000x reference)
"""Trainium2 Bass kernel: sparse (sliding-window) attention block.

Full module per reference:
  RMSNorm -> fused QKV (5120x2880) -> YaRN RoPE -> GQA sliding-window(128)
  causal attention with learned sink logit -> out projection (2880x4096).

Sharding: tensor-parallel over heads across 8 cores. Core c owns q-heads
[8c, 8c+8) and kv-head c. RMSNorm is computed (replicated) on every core.
Each core emits a partial [1024, 2880] output (its heads' out-proj
contribution); the host sums the 8 partials and adds out_b.

Device-side layouts are all pre-arranged on host so every DRAM->SBUF DMA is
contiguous:
  xt    [2880, 1024] bf16   x transposed (hidden on partitions)
  wq    [5, 128, 23*128] bf16  qkv lhsT tiles (n-tile, k-part, k-tile*n-col)
  wout  [4, 128, 2880] bf16    out_w.T shard rhs tiles (hd k-tile, hd-part, H)
  bqkv  [128, 5] f32           qkv bias, n-tile-major
  cos/sin tables [32, 1024] f32 (q tables pre-scaled by sm_scale)
  mask  [128, 256] f32         additive {0,-1e30}: [prev-tile | self-tile]
  esink [128, 8] f32           exp(sinks) broadcast to 128 partitions
"""

import math
import sys

import numpy as np

try:
    import concourse.bass as bass
except ImportError:  # pragma: no cover
    sys.path.insert(0, "/opt/trn_rl_repo")
    import concourse.bass as bass

import concourse.bacc as bacc
import concourse.tile as tile
from concourse import mybir
from concourse.masks import make_identity
from concourse.bass_utils import run_bass_kernel_spmd

import ml_dtypes

BF16 = ml_dtypes.bfloat16

T = 1024
HIDDEN = 2880
HD = 64
NH = 64
NKV = 8
SW = 128
NCORES = 8
HPC = NH // NCORES          # q heads per core = 8
QKV_DIM = HD * (NH + 2 * NKV)
SM_SCALE = 1.0 / math.sqrt(HD)

P = 128
KT = (HIDDEN + P - 1) // P   # 23 k-tiles over hidden (22*128 + 64)
KLAST = HIDDEN - (KT - 1) * P  # 64
NT = 5                       # qkv n-tiles of 128 (4 q-tiles + 1 kv-tile)
MT = T // P                  # 8 token tiles
NEG = -1.0e30

dt = mybir.dt

_CACHE = {}


# ----------------------------------------------------------------------------
# host-side helpers
# ----------------------------------------------------------------------------

def _rope_cos_sin(num_tokens):
    base = 150000.0
    scaling = 32.0
    init_ctx = 4096.0
    ntk_alpha = 1.0
    ntk_beta = 32.0
    d_half = HD / 2
    freq = base ** (np.arange(0, HD, 2, dtype=np.float32) / HD)
    concentration = 0.1 * math.log(scaling) + 1.0
    low = d_half * math.log(init_ctx / (ntk_beta * 2 * math.pi)) / math.log(base)
    high = d_half * math.log(init_ctx / (ntk_alpha * 2 * math.pi)) / math.log(base)
    interpolation = 1.0 / (scaling * freq)
    extrapolation = 1.0 / freq
    ramp = (np.arange(int(d_half), dtype=np.float32) - low) / (high - low)
    m = 1.0 - np.clip(ramp, 0.0, 1.0)
    inv_freq = interpolation * (1.0 - m) + extrapolation * m
    t = np.arange(num_tokens, dtype=np.float32)
    freqs = t[:, None] * inv_freq[None, :]
    cos = (np.cos(freqs) * concentration).astype(np.float32)
    sin = (np.sin(freqs) * concentration).astype(np.float32)
    return cos, sin  # [T, 32]


def _host_masks():
    j = np.arange(P)[:, None]   # kt row (partition)
    i = np.arange(P)[None, :]   # q col (free)
    mask_prev = np.where(j > i, 0.0, NEG).astype(np.float32)   # dist in [1,127]
    mask_self = np.where(j <= i, 0.0, NEG).astype(np.float32)  # dist in [0,127]
    return np.concatenate([mask_prev, mask_self], axis=1)  # [128, 256]


def _prep_core_inputs(core, x, norm_scale, qkv_w, qkv_b, out_w, sinks):
    """Build the per-core input map (all numpy, layouts per module docstring)."""
    q_end = NH * HD
    k_end = q_end + NKV * HD

    # rows of qkv_w for this core: 8 q heads + 1 k head + 1 v head = 640 rows
    qrows = np.arange(core * HPC * HD, (core + 1) * HPC * HD)
    krows = np.arange(q_end + core * HD, q_end + (core + 1) * HD)
    vrows = np.arange(k_end + core * HD, k_end + (core + 1) * HD)
    # kv n-tile holds v in partitions 0:64 (base-0 for the PE transpose) and
    # k in partitions 64:128.
    rows = np.concatenate([qrows, vrows, krows])  # [640]

    wshard = (qkv_w[rows, :] * norm_scale[None, :]).astype(np.float32)  # [640, 2880]
    bshard = qkv_b[rows].astype(np.float32)  # [640]

    # lhsT tiles: wq[n, kp, kt*128 + nc] = wshard[n*128 + nc, kt*128 + kp]
    wq = np.zeros((NT, P, KT * P), dtype=BF16)
    for n in range(NT):
        blk = wshard[n * P:(n + 1) * P, :]  # [128 n, 2880 k]
        for ki in range(KT):
            k0 = ki * P
            ksz = min(P, HIDDEN - k0)
            wq[n, :ksz, ki * P:ki * P + P] = blk[:, k0:k0 + ksz].T.astype(BF16)

    # out_w shard: columns for this core's heads -> [2880, 512] -> T -> [512, 2880]
    cols = np.arange(core * HPC * HD, (core + 1) * HPC * HD)
    wo = out_w[:, cols].T.astype(np.float32)  # [512 hd, 2880 H]
    wout = wo.reshape(4, P, HIDDEN).astype(BF16)

    bqkv = bshard.reshape(NT, P).T.copy().astype(np.float32)  # [128, 5]

    cos, sin = _rope_cos_sin(T)  # [1024, 32]
    # replicated 4x along partitions so 32-aligned slices at any base
    # partition line up with q/k slices (walrus SB-SB base-partition rule)
    # rope on device: out[p] = src[p]*cos[p%32] + src[p^32]*sin_alt[p]
    # where sin_alt carries the -/+ sign for the x1/x2 half of each head.
    sgn = np.repeat([-1.0, 1.0], 32)[:, None].astype(np.float32)
    sgn = np.tile(sgn, (2, 1))  # [128, 1]
    cosq = np.tile(cos.T * SM_SCALE, (4, 1)).astype(np.float32)  # [128, 1024]
    sinq = (np.tile(sin.T * SM_SCALE, (4, 1)) * sgn).astype(np.float32)
    cosk = np.tile(cos.T, (4, 1)).astype(np.float32)
    sink = (np.tile(sin.T, (4, 1)) * sgn).astype(np.float32)

    esink = np.exp(sinks[core * HPC:(core + 1) * HPC].astype(np.float64))
    esink = np.broadcast_to(esink.astype(np.float32), (P, HPC)).copy()

    return {
        "xt": x.T.astype(BF16).copy(),          # [2880, 1024]
        "wq": wq,
        "wout": wout,
        "bqkv": bqkv,
        "cosq": cosq, "sinq": sinq, "cosk": cosk, "sink_t": sink,
        "mask": _host_masks(),
        "esink": esink,
    }


# ----------------------------------------------------------------------------
# device kernel (Tile)
# ----------------------------------------------------------------------------

def build_nc():
    nc = bacc.Bacc("TRN2", target_bir_lowering=False, debug=False)

    xt_d = nc.dram_tensor("xt", [HIDDEN, T], dt.bfloat16, kind="ExternalInput").ap()
    wq_d = nc.dram_tensor("wq", [NT, P, KT * P], dt.bfloat16, kind="ExternalInput").ap()
    wout_d = nc.dram_tensor("wout", [4, P, HIDDEN], dt.bfloat16, kind="ExternalInput").ap()
    bqkv_d = nc.dram_tensor("bqkv", [P, NT], dt.float32, kind="ExternalInput").ap()
    cosq_d = nc.dram_tensor("cosq", [P, T], dt.float32, kind="ExternalInput").ap()
    sinq_d = nc.dram_tensor("sinq", [P, T], dt.float32, kind="ExternalInput").ap()
    cosk_d = nc.dram_tensor("cosk", [P, T], dt.float32, kind="ExternalInput").ap()
    sink_d = nc.dram_tensor("sink_t", [P, T], dt.float32, kind="ExternalInput").ap()
    mask_d = nc.dram_tensor("mask", [P, 2 * P], dt.float32, kind="ExternalInput").ap()
    esink_d = nc.dram_tensor("esink", [P, HPC], dt.float32, kind="ExternalInput").ap()
    y_d = nc.dram_tensor("y", [T, HIDDEN], dt.float32, kind="ExternalOutput").ap()

    YC = 480                     # out-proj psum chunk width (6 chunks of 480)
    AW = HD + 1                  # per-head AV width (64 v-dims + denominator)

    def bcast_free(ap2d, n):
        """[P, F] -> [P, n, F] with a 0-step middle dim (free broadcast)."""
        return bass.AP(tensor=ap2d.tensor, offset=ap2d.offset,
                       ap=[ap2d.ap[0], [0, n]] + list(ap2d.ap[1:]))

    with tile.TileContext(nc) as tc:
        with (
            tc.tile_pool(name="const", bufs=1) as const,
            tc.tile_pool(name="res", bufs=1) as res,
            tc.tile_pool(name="xsqp", bufs=3) as xsqp,
            tc.tile_pool(name="ptp", bufs=3) as ptp,
            tc.tile_pool(name="anormp", bufs=3) as anormp,
            tc.tile_pool(name="atp", bufs=6) as atp,
            tc.tile_pool(name="smallp", bufs=4) as smallp,
            tc.tile_pool(name="ropep", bufs=2) as ropep,
            tc.tile_pool(name="ysbp", bufs=3) as ysbp,
            tc.tile_pool(name="pb1", bufs=2, space="PSUM") as pb1,
            tc.tile_pool(name="pscore", bufs=2, space="PSUM") as pscore,
            tc.tile_pool(name="psmall", bufs=2, space="PSUM") as psmall,
        ):
            # ---- constants / parameters ----
            ones = const.tile([P, 1], dt.bfloat16, tag="ones", name="ones")
            nc.vector.memset(ones, 1.0)
            ident = const.tile([P, P], dt.float32, tag="ident", name="ident")
            make_identity(nc, ident)
            zbias = const.tile([P, 1], dt.float32, tag="zbias", name="zbias")
            nc.vector.memset(zbias, 0.0)
            eps_t = const.tile([1, 1], dt.float32, tag="eps", name="eps")
            nc.vector.memset(eps_t, 1e-5)

            # kv-tile qkv weights first: they unblock the first matmuls
            wq_sb = [None] * NT
            w4 = res.tile([P, KT * P], dt.bfloat16, tag="wq4", name="wq4")
            nc.sync.dma_start(out=w4, in_=wq_d[4])
            wq_sb[4] = w4
            # ---- x transposed, resident; squares + ssq accumulation ----
            xt_sb = res.tile([P, KT, T], dt.bfloat16, tag="xt", name="xt")
            psum_ssq = pscore.tile([1, T], dt.float32, tag="score", name="ssq")
            for ki in range(KT):
                k0 = ki * P
                ksz = min(P, HIDDEN - k0)
                eng = nc.sync if ki % 2 == 0 else nc.gpsimd
                eng.dma_start(out=xt_sb[:ksz, ki, :], in_=xt_d[k0:k0 + ksz, :])
                xsq = xsqp.tile([P, T], dt.bfloat16, tag="xsq", name="xsq")
                nc.scalar.activation(xsq[:ksz], xt_sb[:ksz, ki, :],
                                     mybir.ActivationFunctionType.Square,
                                     bias=zbias[:ksz, :])
                for half in range(2):
                    c0 = half * 512
                    nc.tensor.matmul(
                        psum_ssq[:, c0:c0 + 512],
                        ones[:ksz, :],
                        xsq[:ksz, c0:c0 + 512],
                        start=(ki == 0), stop=(ki == KT - 1),
                    )

            # deferred loads: q-tile weights, rope tables, small consts
            for n in (0, 1, 2, 3):
                w = res.tile([P, KT * P], dt.bfloat16, tag=f"wq{n}", name=f"wq{n}")
                nc.sync.dma_start(out=w, in_=wq_d[n])
                wq_sb[n] = w
            tabs = {}
            for nm, d in (("cosq", cosq_d), ("sinq", sinq_d),
                          ("cosk", cosk_d), ("sink_t", sink_d)):
                tabs[nm] = const.tile([P, T], dt.float32, tag=nm, name=nm)
                nc.sync.dma_start(out=tabs[nm], in_=d)
            mask_sb = const.tile([P, 2 * P], dt.float32, tag="mask", name="mask")
            nc.sync.dma_start(out=mask_sb, in_=mask_d)
            esink_sb = const.tile([P, HPC], dt.float32, tag="esink", name="esink")
            nc.sync.dma_start(out=esink_sb, in_=esink_d)
            bqkv_sb = const.tile([P, NT], dt.float32, tag="bqkv", name="bqkv")
            nc.sync.dma_start(out=bqkv_sb, in_=bqkv_d)

            # rsqrt(mean + eps) = exp(-0.5 * ln(ssq/H + eps)); broadcast to 128 rows
            lnm = res.tile([1, T], dt.float32, tag="lnm", name="lnm")
            nc.scalar.activation(lnm, psum_ssq, mybir.ActivationFunctionType.Ln,
                                 bias=eps_t, scale=1.0 / HIDDEN)
            rinv = res.tile([1, T], dt.float32, tag="rinv", name="rinv")
            nc.scalar.activation(rinv, lnm, mybir.ActivationFunctionType.Exp,
                                 bias=zbias[:1, :], scale=-0.5)
            rsq_b = res.tile([P, T], dt.float32, tag="rsq_b", name="rsq_b")
            nc.gpsimd.partition_broadcast(rsq_b, rinv)

            # ---- QKV matmuls + post-scale (kv tile first so rope-k / v
            # transposes / scores can start while q tiles are still computing)
            qkvT = []
            for n in range(NT):
                dtype = dt.float32 if n == NT - 1 else dt.bfloat16
                qkvT.append(res.tile([P, T], dtype, tag=f"qkvT{n}", name=f"qkvT{n}"))

            def qkv_tile(n):
                for half in range(2):
                    c0 = half * 512
                    pq = pb1.tile([P, 512], dt.float32, tag="pb", name="pb")
                    for ki in range(KT):
                        ksz = min(P, HIDDEN - ki * P)
                        nc.tensor.matmul(
                            pq,
                            wq_sb[n][:ksz, ki * P:ki * P + P],
                            xt_sb[:ksz, ki, c0:c0 + 512],
                            start=(ki == 0), stop=(ki == KT - 1),
                        )
                    # drain psum immediately (no rsqrt dependency), then
                    # apply (* rsqrt + bias) in place
                    nc.vector.tensor_copy(qkvT[n][:, c0:c0 + 512], pq)
                    nc.vector.tensor_mul(qkvT[n][:, c0:c0 + 512],
                                         qkvT[n][:, c0:c0 + 512],
                                         rsq_b[:, c0:c0 + 512])
                    nc.vector.tensor_scalar_add(qkvT[n][:, c0:c0 + 512],
                                                qkvT[n][:, c0:c0 + 512],
                                                bqkv_sb[:, n:n + 1])

            # ---- RoPE: swap-copy + two full-width muls + per-head add ----
            def rope(srct, lo, hi, dsts, cos_t, sin_t, kdt):
                # rows lo:hi of srct hold n heads (64 rows each); for each
                # row p: out[p] = src[p]*cos[p] + src[p^32]*sin_alt[p]
                swp = ropep.tile([P, T], kdt, tag="swp", name="swp")
                for a in range(lo, hi, 32):
                    b = a ^ 32
                    nc.vector.tensor_copy(swp[a:a + 32, :], srct[b:b + 32, :])
                tc = ropep.tile([P, T], dt.float32, tag="tc", name="tc")
                nc.vector.tensor_mul(tc[lo:hi, :], srct[lo:hi, :],
                                     cos_t[lo:hi, :])
                ts = ropep.tile([P, T], dt.float32, tag="ts", name="ts")
                nc.vector.tensor_mul(ts[lo:hi, :], swp[lo:hi, :],
                                     sin_t[lo:hi, :])
                for i, dst in enumerate(dsts):
                    b0 = lo + 64 * i
                    nc.vector.tensor_add(dst, tc[b0:b0 + 64, :],
                                         ts[b0:b0 + 64, :])

            qra = res.tile([64, HPC, T], dt.bfloat16, tag="qra", name="qra")
            krope = res.tile([64, T], dt.bfloat16, tag="krope", name="krope")

            # kv first: krope + token-major v (+ ones column) unblock attention
            qkv_tile(4)
            rope(qkvT[4], 64, 128, [krope], tabs["cosk"], tabs["sink_t"],
                 dt.float32)
            vtok = []
            for b in range(MT):
                pv = psmall.tile([P, 64], dt.float32, tag="small", name="small")
                nc.tensor.transpose(pv, qkvT[4][0:64, b * P:(b + 1) * P],
                                    ident[:64, :64])
                vt = res.tile([P, AW], dt.bfloat16, tag=f"vtok{b}", name=f"vtok{b}")
                nc.vector.tensor_copy(vt[:, 0:HD], pv)
                nc.vector.memset(vt[:, HD:HD + 1], 1.0)
                vtok.append(vt)

            for n in range(4):
                qkv_tile(n)
                rope(qkvT[n], 0, 128, [qra[:, 2 * n, :], qra[:, 2 * n + 1, :]],
                     tabs["cosq"], tabs["sinq"], dt.bfloat16)

            # out-proj weights: issue loads late so they don't delay xt/wq
            wout_sb = []
            for kk in range(4):
                w = res.tile([P, HIDDEN], dt.bfloat16, tag=f"wout{kk}", name=f"wout{kk}")
                nc.sync.dma_start(out=w, in_=wout_d[kk])
                wout_sb.append(w)

            # ---- attention (all 8 heads batched per token tile) ----
            for b in range(MT):
                # scores vs prev / self kt tile; [128 kt, 8 heads, 128 q]
                pts = []
                for kt, m0 in ((b - 1, 0), (b, P)):
                    if kt < 0:
                        pts.append(None)
                        continue
                    ps = pscore.tile([P, HPC, P], dt.float32, tag="score",
                                     name="score")
                    for g in range(2):
                        nc.tensor.matmul(
                            ps[:, 4 * g:4 * g + 4, :],
                            krope[:, kt * P:(kt + 1) * P],
                            qra[:, 4 * g:4 * g + 4, b * P:(b + 1) * P],
                            start=True, stop=True)
                    nc.vector.tensor_add(ps, ps,
                                         bcast_free(mask_sb[:, m0:m0 + P], HPC))
                    pt = ptp.tile([P, HPC, P], dt.bfloat16, tag="pt", name="pt")
                    nc.scalar.activation(pt, ps,
                                         mybir.ActivationFunctionType.Exp,
                                         bias=zbias)
                    pts.append(pt)
                ptA, ptB = pts

                # AV per head into two 4-head psum groups; batched normalize
                rec8 = smallp.tile([P, HPC], dt.float32, tag="rec8", name="rec8")
                anorm = []
                for g in range(2):
                    pg = psmall.tile([P, 4, AW], dt.float32, tag="small",
                                     name="small")
                    for j in range(4):
                        h = 4 * g + j
                        if b > 0:
                            nc.tensor.matmul(pg[:, j, :], ptA[:, h, :],
                                             vtok[b - 1], start=True, stop=False)
                            nc.tensor.matmul(pg[:, j, :], ptB[:, h, :],
                                             vtok[b], start=False, stop=True)
                        else:
                            nc.tensor.matmul(pg[:, j, :], ptB[:, h, :],
                                             vtok[b], start=True, stop=True)
                    g0 = 4 * g
                    nc.vector.tensor_add(rec8[:, g0:g0 + 4],
                                         pg[:, :, HD:HD + 1],
                                         esink_sb[:, g0:g0 + 4])
                    nc.vector.reciprocal(rec8[:, g0:g0 + 4], rec8[:, g0:g0 + 4])
                    an = anormp.tile([P, 4, HD], dt.float32, tag="anorm",
                                     name="anorm")
                    rec3 = bass.AP(tensor=rec8.tensor, offset=rec8[:, g0:g0 + 4].offset,
                                   ap=[rec8.ap[0], [1, 4], [0, HD]])
                    nc.vector.tensor_mul(an, pg[:, :, 0:HD], rec3)
                    anorm.append(an)

                # transpose to head-major [128 hd, 128 tok] bf16 tiles
                att = []
                for g in range(2):
                    a2 = anorm[g].rearrange("p a b -> p (a b)")
                    for j in range(2):
                        pat = psmall.tile([P, P], dt.float32, tag="small",
                                          name="small")
                        nc.tensor.transpose(pat, a2[:, j * P:(j + 1) * P], ident)
                        at = atp.tile([P, P], dt.bfloat16, tag="at", name="at")
                        nc.any.tensor_copy(at, pat)
                        att.append(at)

                # out projection: y[b*128:(b+1)*128, :] partial
                for ch in range(HIDDEN // YC):
                    o0 = ch * YC
                    py = pb1.tile([P, YC], dt.float32, tag="pb", name="pb")
                    for kk in range(4):
                        nc.tensor.matmul(py, att[kk],
                                         wout_sb[kk][:, o0:o0 + YC],
                                         start=(kk == 0), stop=(kk == 3))
                    ysb = ysbp.tile([P, YC], dt.float32, tag="ysb", name="ysb")
                    nc.any.tensor_copy(ysb, py)
                    nc.sync.dma_start(out=y_d[b * P:(b + 1) * P, o0:o0 + YC],
                                      in_=ysb)

    nc.compile()
    return nc


# ----------------------------------------------------------------------------
# public entry
# ----------------------------------------------------------------------------

LAST_RESULTS = None


def kernel(x, norm_scale, qkv_w, qkv_b, out_w, out_b, sinks):
    global LAST_RESULTS
    x = np.asarray(x, dtype=np.float32)
    norm_scale = np.asarray(norm_scale, dtype=np.float32)
    qkv_w = np.asarray(qkv_w, dtype=np.float32)
    qkv_b = np.asarray(qkv_b, dtype=np.float32)
    out_w = np.asarray(out_w, dtype=np.float32)
    out_b = np.asarray(out_b, dtype=np.float32)
    sinks = np.asarray(sinks, dtype=np.float32)

    if "nc" not in _CACHE:
        _CACHE["nc"] = build_nc()
    nc = _CACHE["nc"]

    in_maps = [
        _prep_core_inputs(c, x, norm_scale, qkv_w, qkv_b, out_w, sinks)
        for c in range(NCORES)
    ]
    import os
    tmpdir = os.environ.get("BASS_TMPDIR") or None
    res = run_bass_kernel_spmd(nc, in_maps, core_ids=list(range(NCORES)),
                               tmpdir=tmpdir)
    LAST_RESULTS = res
    y = np.zeros((T, HIDDEN), dtype=np.float64)
    for c in range(NCORES):
        y += res.results[c]["y"].astype(np.float64)
    y += out_b.astype(np.float64)[None, :]
    return y.astype(np.float32)


# revision 21
# speedup vs baseline: 1.0484x; 1.0484x over previous
"""Trainium2 Bass kernel: sparse (sliding-window) attention block.

Full module per reference:
  RMSNorm -> fused QKV (5120x2880) -> YaRN RoPE -> GQA sliding-window(128)
  causal attention with learned sink logit -> out projection (2880x4096).

Sharding: tensor-parallel over heads across 8 cores. Core c owns q-heads
[8c, 8c+8) and kv-head c. RMSNorm is computed (replicated) on every core.
Each core emits a partial [1024, 2880] output (its heads' out-proj
contribution); the host sums the 8 partials and adds out_b.

Device-side layouts are all pre-arranged on host so every DRAM->SBUF DMA is
contiguous:
  xt    [2880, 1024] bf16   x transposed (hidden on partitions)
  wq    [5, 128, 23*128] bf16  qkv lhsT tiles (n-tile, k-part, k-tile*n-col)
  wout  [4, 128, 2880] bf16    out_w.T shard rhs tiles (hd k-tile, hd-part, H)
  bqkv  [128, 5] f32           qkv bias, n-tile-major
  cos/sin tables [32, 1024] f32 (q tables pre-scaled by sm_scale)
  mask  [128, 256] f32         additive {0,-1e30}: [prev-tile | self-tile]
  esink [128, 8] f32           exp(sinks) broadcast to 128 partitions
"""

import math
import sys

import numpy as np

try:
    import concourse.bass as bass
except ImportError:  # pragma: no cover
    sys.path.insert(0, "/opt/trn_rl_repo")
    import concourse.bass as bass

import concourse.bacc as bacc
import concourse.tile as tile
from concourse import mybir
from concourse.masks import make_identity
from concourse.bass_utils import run_bass_kernel_spmd

import ml_dtypes

BF16 = ml_dtypes.bfloat16

T = 1024
HIDDEN = 2880
HD = 64
NH = 64
NKV = 8
SW = 128
NCORES = 8
HPC = NH // NCORES          # q heads per core = 8
QKV_DIM = HD * (NH + 2 * NKV)
SM_SCALE = 1.0 / math.sqrt(HD)

P = 128
KT = (HIDDEN + P - 1) // P   # 23 k-tiles over hidden (22*128 + 64)
KLAST = HIDDEN - (KT - 1) * P  # 64
NT = 5                       # qkv n-tiles of 128 (4 q-tiles + 1 kv-tile)
MT = T // P                  # 8 token tiles
NEG = -1.0e30

dt = mybir.dt

_CACHE = {}


# ----------------------------------------------------------------------------
# host-side helpers
# ----------------------------------------------------------------------------

def _rope_cos_sin(num_tokens):
    base = 150000.0
    scaling = 32.0
    init_ctx = 4096.0
    ntk_alpha = 1.0
    ntk_beta = 32.0
    d_half = HD / 2
    freq = base ** (np.arange(0, HD, 2, dtype=np.float32) / HD)
    concentration = 0.1 * math.log(scaling) + 1.0
    low = d_half * math.log(init_ctx / (ntk_beta * 2 * math.pi)) / math.log(base)
    high = d_half * math.log(init_ctx / (ntk_alpha * 2 * math.pi)) / math.log(base)
    interpolation = 1.0 / (scaling * freq)
    extrapolation = 1.0 / freq
    ramp = (np.arange(int(d_half), dtype=np.float32) - low) / (high - low)
    m = 1.0 - np.clip(ramp, 0.0, 1.0)
    inv_freq = interpolation * (1.0 - m) + extrapolation * m
    t = np.arange(num_tokens, dtype=np.float32)
    freqs = t[:, None] * inv_freq[None, :]
    cos = (np.cos(freqs) * concentration).astype(np.float32)
    sin = (np.sin(freqs) * concentration).astype(np.float32)
    return cos, sin  # [T, 32]


def _host_masks():
    j = np.arange(P)[:, None]   # kt row (partition)
    i = np.arange(P)[None, :]   # q col (free)
    mask_prev = np.where(j > i, 0.0, NEG).astype(np.float32)   # dist in [1,127]
    mask_self = np.where(j <= i, 0.0, NEG).astype(np.float32)  # dist in [0,127]
    return np.concatenate([mask_prev, mask_self], axis=1)  # [128, 256]


def _prep_core_inputs(core, x, norm_scale, qkv_w, qkv_b, out_w, sinks):
    """Build the per-core input map (all numpy, layouts per module docstring)."""
    q_end = NH * HD
    k_end = q_end + NKV * HD

    # rows of qkv_w for this core: 8 q heads + 1 k head + 1 v head = 640 rows
    qrows = np.arange(core * HPC * HD, (core + 1) * HPC * HD)
    krows = np.arange(q_end + core * HD, q_end + (core + 1) * HD)
    vrows = np.arange(k_end + core * HD, k_end + (core + 1) * HD)
    # kv n-tile holds v in partitions 0:64 (base-0 for the PE transpose) and
    # k in partitions 64:128.
    rows = np.concatenate([qrows, vrows, krows])  # [640]

    wshard = (qkv_w[rows, :] * norm_scale[None, :]).astype(np.float32)  # [640, 2880]
    bshard = qkv_b[rows].astype(np.float32)  # [640]

    # lhsT tiles: wq[n, kp, kt*128 + nc] = wshard[n*128 + nc, kt*128 + kp]
    wq = np.zeros((NT, P, KT * P), dtype=BF16)
    for n in range(NT):
        blk = wshard[n * P:(n + 1) * P, :]  # [128 n, 2880 k]
        for ki in range(KT):
            k0 = ki * P
            ksz = min(P, HIDDEN - k0)
            wq[n, :ksz, ki * P:ki * P + P] = blk[:, k0:k0 + ksz].T.astype(BF16)

    # out_w shard: columns for this core's heads -> [2880, 512] -> T -> [512, 2880]
    cols = np.arange(core * HPC * HD, (core + 1) * HPC * HD)
    wo = out_w[:, cols].T.astype(np.float32)  # [512 hd, 2880 H]
    wout = wo.reshape(4, P, HIDDEN).astype(BF16)

    bqkv = bshard.reshape(NT, P).T.copy().astype(np.float32)  # [128, 5]

    cos, sin = _rope_cos_sin(T)  # [1024, 32]
    # replicated 4x along partitions so 32-aligned slices at any base
    # partition line up with q/k slices (walrus SB-SB base-partition rule)
    # rope on device: out[p] = src[p]*cos[p%32] + src[p^32]*sin_alt[p]
    # where sin_alt carries the -/+ sign for the x1/x2 half of each head.
    sgn = np.repeat([-1.0, 1.0], 32)[:, None].astype(np.float32)
    sgn = np.tile(sgn, (2, 1))  # [128, 1]
    cosq = np.tile(cos.T * SM_SCALE, (4, 1)).astype(np.float32)  # [128, 1024]
    sinq = (np.tile(sin.T * SM_SCALE, (4, 1)) * sgn).astype(np.float32)
    cosk = np.tile(cos.T, (4, 1)).astype(np.float32)
    sink = (np.tile(sin.T, (4, 1)) * sgn).astype(np.float32)

    esink = np.exp(sinks[core * HPC:(core + 1) * HPC].astype(np.float64))
    esink = np.broadcast_to(esink.astype(np.float32), (P, HPC)).copy()

    return {
        "xt": x.T.astype(BF16).copy(),          # [2880, 1024]
        "wq": wq,
        "wout": wout,
        "bqkv": bqkv,
        "cosq": cosq, "sinq": sinq, "cosk": cosk, "sink_t": sink,
        "mask": _host_masks(),
        "esink": esink,
    }


# ----------------------------------------------------------------------------
# device kernel (Tile)
# ----------------------------------------------------------------------------

def build_nc():
    nc = bacc.Bacc("TRN2", target_bir_lowering=False, debug=False)

    xt_d = nc.dram_tensor("xt", [HIDDEN, T], dt.bfloat16, kind="ExternalInput").ap()
    wq_d = nc.dram_tensor("wq", [NT, P, KT * P], dt.bfloat16, kind="ExternalInput").ap()
    wout_d = nc.dram_tensor("wout", [4, P, HIDDEN], dt.bfloat16, kind="ExternalInput").ap()
    bqkv_d = nc.dram_tensor("bqkv", [P, NT], dt.float32, kind="ExternalInput").ap()
    cosq_d = nc.dram_tensor("cosq", [P, T], dt.float32, kind="ExternalInput").ap()
    sinq_d = nc.dram_tensor("sinq", [P, T], dt.float32, kind="ExternalInput").ap()
    cosk_d = nc.dram_tensor("cosk", [P, T], dt.float32, kind="ExternalInput").ap()
    sink_d = nc.dram_tensor("sink_t", [P, T], dt.float32, kind="ExternalInput").ap()
    mask_d = nc.dram_tensor("mask", [P, 2 * P], dt.float32, kind="ExternalInput").ap()
    esink_d = nc.dram_tensor("esink", [P, HPC], dt.float32, kind="ExternalInput").ap()
    y_d = nc.dram_tensor("y", [T, HIDDEN], dt.float32, kind="ExternalOutput").ap()

    YC = 480                     # out-proj psum chunk width (6 chunks of 480)
    AW = HD + 1                  # per-head AV width (64 v-dims + denominator)

    def bcast_free(ap2d, n):
        """[P, F] -> [P, n, F] with a 0-step middle dim (free broadcast)."""
        return bass.AP(tensor=ap2d.tensor, offset=ap2d.offset,
                       ap=[ap2d.ap[0], [0, n]] + list(ap2d.ap[1:]))

    with tile.TileContext(nc) as tc:
        with (
            tc.tile_pool(name="const", bufs=1) as const,
            tc.tile_pool(name="res", bufs=1) as res,
            tc.tile_pool(name="xsqp", bufs=3) as xsqp,
            tc.tile_pool(name="ptp", bufs=3) as ptp,
            tc.tile_pool(name="anormp", bufs=3) as anormp,
            tc.tile_pool(name="atp", bufs=6) as atp,
            tc.tile_pool(name="smallp", bufs=4) as smallp,
            tc.tile_pool(name="ropep", bufs=2) as ropep,
            tc.tile_pool(name="ysbp", bufs=3) as ysbp,
            tc.tile_pool(name="pb1", bufs=2, space="PSUM") as pb1,
            tc.tile_pool(name="pscore", bufs=2, space="PSUM") as pscore,
            tc.tile_pool(name="psmall", bufs=2, space="PSUM") as psmall,
        ):
            # ---- constants / parameters ----
            ones = const.tile([P, 1], dt.bfloat16, tag="ones", name="ones")
            nc.vector.memset(ones, 1.0)
            ident = const.tile([P, P], dt.float32, tag="ident", name="ident")
            make_identity(nc, ident)
            zbias = const.tile([P, 1], dt.float32, tag="zbias", name="zbias")
            nc.vector.memset(zbias, 0.0)
            eps_t = const.tile([1, 1], dt.float32, tag="eps", name="eps")
            nc.vector.memset(eps_t, 1e-5)

            # kv-tile qkv weights first: they unblock the first matmuls
            wq_sb = [None] * NT
            w4 = res.tile([P, KT * P], dt.bfloat16, tag="wq4", name="wq4")
            nc.sync.dma_start(out=w4, in_=wq_d[4])
            wq_sb[4] = w4
            # ---- x transposed, resident; squares + ssq accumulation ----
            xt_sb = res.tile([P, KT, T], dt.bfloat16, tag="xt", name="xt")
            psum_ssq = pscore.tile([1, T], dt.float32, tag="score", name="ssq")
            for ki in range(KT):
                k0 = ki * P
                ksz = min(P, HIDDEN - k0)
                nc.sync.dma_start(out=xt_sb[:ksz, ki, :], in_=xt_d[k0:k0 + ksz, :])
                xsq = xsqp.tile([P, T], dt.bfloat16, tag="xsq", name="xsq")
                nc.scalar.activation(xsq[:ksz], xt_sb[:ksz, ki, :],
                                     mybir.ActivationFunctionType.Square,
                                     bias=zbias[:ksz, :])
                for half in range(2):
                    c0 = half * 512
                    nc.tensor.matmul(
                        psum_ssq[:, c0:c0 + 512],
                        ones[:ksz, :],
                        xsq[:ksz, c0:c0 + 512],
                        start=(ki == 0), stop=(ki == KT - 1),
                    )

            # deferred loads: q-tile weights, rope tables, small consts
            for n in (0, 1, 2, 3):
                w = res.tile([P, KT * P], dt.bfloat16, tag=f"wq{n}", name=f"wq{n}")
                nc.sync.dma_start(out=w, in_=wq_d[n])
                wq_sb[n] = w
            tabs = {}
            for nm, d in (("cosq", cosq_d), ("sinq", sinq_d),
                          ("cosk", cosk_d), ("sink_t", sink_d)):
                tabs[nm] = const.tile([P, T], dt.float32, tag=nm, name=nm)
                nc.sync.dma_start(out=tabs[nm], in_=d)
            mask_sb = const.tile([P, 2 * P], dt.float32, tag="mask", name="mask")
            nc.sync.dma_start(out=mask_sb, in_=mask_d)
            esink_sb = const.tile([P, HPC], dt.float32, tag="esink", name="esink")
            nc.sync.dma_start(out=esink_sb, in_=esink_d)
            bqkv_sb = const.tile([P, NT], dt.float32, tag="bqkv", name="bqkv")
            nc.sync.dma_start(out=bqkv_sb, in_=bqkv_d)

            # rsqrt(mean + eps) = exp(-0.5 * ln(ssq/H + eps)); broadcast to 128 rows
            lnm = res.tile([1, T], dt.float32, tag="lnm", name="lnm")
            nc.scalar.activation(lnm, psum_ssq, mybir.ActivationFunctionType.Ln,
                                 bias=eps_t, scale=1.0 / HIDDEN)
            rinv = res.tile([1, T], dt.float32, tag="rinv", name="rinv")
            nc.scalar.activation(rinv, lnm, mybir.ActivationFunctionType.Exp,
                                 bias=zbias[:1, :], scale=-0.5)
            rsq_b = res.tile([P, T], dt.float32, tag="rsq_b", name="rsq_b")
            nc.gpsimd.partition_broadcast(rsq_b, rinv)

            # ---- QKV matmuls + post-scale (kv tile first so rope-k / v
            # transposes / scores can start while q tiles are still computing)
            qkvT = []
            for n in range(NT):
                dtype = dt.float32 if n == NT - 1 else dt.bfloat16
                qkvT.append(res.tile([P, T], dtype, tag=f"qkvT{n}", name=f"qkvT{n}"))

            def qkv_tile(n):
                for half in range(2):
                    c0 = half * 512
                    pq = pb1.tile([P, 512], dt.float32, tag="pb", name="pb")
                    for ki in range(KT):
                        ksz = min(P, HIDDEN - ki * P)
                        nc.tensor.matmul(
                            pq,
                            wq_sb[n][:ksz, ki * P:ki * P + P],
                            xt_sb[:ksz, ki, c0:c0 + 512],
                            start=(ki == 0), stop=(ki == KT - 1),
                        )
                    # (raw * rsqrt) + bias  -> sbuf (bf16 q tiles, f32 kv)
                    nc.vector.tensor_mul(qkvT[n][:, c0:c0 + 512], pq,
                                         rsq_b[:, c0:c0 + 512])
                    nc.vector.tensor_scalar_add(qkvT[n][:, c0:c0 + 512],
                                                qkvT[n][:, c0:c0 + 512],
                                                bqkv_sb[:, n:n + 1])

            # ---- RoPE: swap-copy + two full-width muls + per-head add ----
            def rope(srct, lo, hi, dsts, cos_t, sin_t, kdt):
                # rows lo:hi of srct hold n heads (64 rows each); for each
                # row p: out[p] = src[p]*cos[p] + src[p^32]*sin_alt[p]
                swp = ropep.tile([P, T], kdt, tag="swp", name="swp")
                for a in range(lo, hi, 32):
                    b = a ^ 32
                    nc.vector.tensor_copy(swp[a:a + 32, :], srct[b:b + 32, :])
                tc = ropep.tile([P, T], dt.float32, tag="tc", name="tc")
                nc.vector.tensor_mul(tc[lo:hi, :], srct[lo:hi, :],
                                     cos_t[lo:hi, :])
                ts = ropep.tile([P, T], dt.float32, tag="ts", name="ts")
                nc.vector.tensor_mul(ts[lo:hi, :], swp[lo:hi, :],
                                     sin_t[lo:hi, :])
                for i, dst in enumerate(dsts):
                    b0 = lo + 64 * i
                    nc.vector.tensor_add(dst, tc[b0:b0 + 64, :],
                                         ts[b0:b0 + 64, :])

            qra = res.tile([64, HPC, T], dt.bfloat16, tag="qra", name="qra")
            krope = res.tile([64, T], dt.bfloat16, tag="krope", name="krope")

            # kv first: krope + token-major v (+ ones column) unblock attention
            qkv_tile(4)
            rope(qkvT[4], 64, 128, [krope], tabs["cosk"], tabs["sink_t"],
                 dt.float32)
            vtok = []
            for b in range(MT):
                pv = psmall.tile([P, 64], dt.float32, tag="small", name="small")
                nc.tensor.transpose(pv, qkvT[4][0:64, b * P:(b + 1) * P],
                                    ident[:64, :64])
                vt = res.tile([P, AW], dt.bfloat16, tag=f"vtok{b}", name=f"vtok{b}")
                nc.vector.tensor_copy(vt[:, 0:HD], pv)
                nc.vector.memset(vt[:, HD:HD + 1], 1.0)
                vtok.append(vt)

            for n in range(4):
                qkv_tile(n)
                rope(qkvT[n], 0, 128, [qra[:, 2 * n, :], qra[:, 2 * n + 1, :]],
                     tabs["cosq"], tabs["sinq"], dt.bfloat16)

            # out-proj weights: issue loads late so they don't delay xt/wq
            wout_sb = []
            for kk in range(4):
                w = res.tile([P, HIDDEN], dt.bfloat16, tag=f"wout{kk}", name=f"wout{kk}")
                nc.sync.dma_start(out=w, in_=wout_d[kk])
                wout_sb.append(w)

            # ---- attention (all 8 heads batched per token tile) ----
            for b in range(MT):
                # scores vs prev / self kt tile; [128 kt, 8 heads, 128 q]
                pts = []
                for kt, m0 in ((b - 1, 0), (b, P)):
                    if kt < 0:
                        pts.append(None)
                        continue
                    ps = pscore.tile([P, HPC, P], dt.float32, tag="score",
                                     name="score")
                    for g in range(2):
                        nc.tensor.matmul(
                            ps[:, 4 * g:4 * g + 4, :],
                            krope[:, kt * P:(kt + 1) * P],
                            qra[:, 4 * g:4 * g + 4, b * P:(b + 1) * P],
                            start=True, stop=True)
                    nc.vector.tensor_add(ps, ps,
                                         bcast_free(mask_sb[:, m0:m0 + P], HPC))
                    pt = ptp.tile([P, HPC, P], dt.bfloat16, tag="pt", name="pt")
                    nc.scalar.activation(pt, ps,
                                         mybir.ActivationFunctionType.Exp,
                                         bias=zbias)
                    pts.append(pt)
                ptA, ptB = pts

                # AV per head into two 4-head psum groups; batched normalize
                rec8 = smallp.tile([P, HPC], dt.float32, tag="rec8", name="rec8")
                anorm = []
                for g in range(2):
                    pg = psmall.tile([P, 4, AW], dt.float32, tag="small",
                                     name="small")
                    for j in range(4):
                        h = 4 * g + j
                        if b > 0:
                            nc.tensor.matmul(pg[:, j, :], ptA[:, h, :],
                                             vtok[b - 1], start=True, stop=False)
                            nc.tensor.matmul(pg[:, j, :], ptB[:, h, :],
                                             vtok[b], start=False, stop=True)
                        else:
                            nc.tensor.matmul(pg[:, j, :], ptB[:, h, :],
                                             vtok[b], start=True, stop=True)
                    g0 = 4 * g
                    nc.vector.tensor_add(rec8[:, g0:g0 + 4],
                                         pg[:, :, HD:HD + 1],
                                         esink_sb[:, g0:g0 + 4])
                    nc.vector.reciprocal(rec8[:, g0:g0 + 4], rec8[:, g0:g0 + 4])
                    an = anormp.tile([P, 4, HD], dt.float32, tag="anorm",
                                     name="anorm")
                    rec3 = bass.AP(tensor=rec8.tensor, offset=rec8[:, g0:g0 + 4].offset,
                                   ap=[rec8.ap[0], [1, 4], [0, HD]])
                    nc.vector.tensor_mul(an, pg[:, :, 0:HD], rec3)
                    anorm.append(an)

                # transpose to head-major [128 hd, 128 tok] bf16 tiles
                att = []
                for g in range(2):
                    a2 = anorm[g].rearrange("p a b -> p (a b)")
                    for j in range(2):
                        pat = psmall.tile([P, P], dt.float32, tag="small",
                                          name="small")
                        nc.tensor.transpose(pat, a2[:, j * P:(j + 1) * P], ident)
                        at = atp.tile([P, P], dt.bfloat16, tag="at", name="at")
                        nc.any.tensor_copy(at, pat)
                        att.append(at)

                # out projection: y[b*128:(b+1)*128, :] partial
                for ch in range(HIDDEN // YC):
                    o0 = ch * YC
                    py = pb1.tile([P, YC], dt.float32, tag="pb", name="pb")
                    for kk in range(4):
                        nc.tensor.matmul(py, att[kk],
                                         wout_sb[kk][:, o0:o0 + YC],
                                         start=(kk == 0), stop=(kk == 3))
                    ysb = ysbp.tile([P, YC], dt.float32, tag="ysb", name="ysb")
                    nc.any.tensor_copy(ysb, py)
                    nc.sync.dma_start(out=y_d[b * P:(b + 1) * P, o0:o0 + YC],
                                      in_=ysb)

    nc.compile()
    return nc


# ----------------------------------------------------------------------------
# public entry
# ----------------------------------------------------------------------------

LAST_RESULTS = None


def kernel(x, norm_scale, qkv_w, qkv_b, out_w, out_b, sinks):
    global LAST_RESULTS
    x = np.asarray(x, dtype=np.float32)
    norm_scale = np.asarray(norm_scale, dtype=np.float32)
    qkv_w = np.asarray(qkv_w, dtype=np.float32)
    qkv_b = np.asarray(qkv_b, dtype=np.float32)
    out_w = np.asarray(out_w, dtype=np.float32)
    out_b = np.asarray(out_b, dtype=np.float32)
    sinks = np.asarray(sinks, dtype=np.float32)

    if "nc" not in _CACHE:
        _CACHE["nc"] = build_nc()
    nc = _CACHE["nc"]

    in_maps = [
        _prep_core_inputs(c, x, norm_scale, qkv_w, qkv_b, out_w, sinks)
        for c in range(NCORES)
    ]
    import os
    tmpdir = os.environ.get("BASS_TMPDIR") or None
    res = run_bass_kernel_spmd(nc, in_maps, core_ids=list(range(NCORES)),
                               tmpdir=tmpdir)
    LAST_RESULTS = res
    y = np.zeros((T, HIDDEN), dtype=np.float64)
    for c in range(NCORES):
        y += res.results[c]["y"].astype(np.float64)
    y += out_b.astype(np.float64)[None, :]
    return y.astype(np.float32)
